# revision 2
# baseline (speedup 1.0000x reference)
"""CropAndResize (TF semantics, 32x32 crops, bilinear, extrapolation=0) on 8
Trainium2 NeuronCores via Bass/Tile.

Strategy
--------
Sharding: core (b, h) = 2*b + h owns batch-image b, channel half h (128
channels) and processes every box with box_ind == b (box_ind-aware routing
on host). The 20 MiB image half stays resident in SBUF.

Unlike the per-core-compiled-constants design, the kernel is DATA-DRIVEN:
one single SPMD program runs on all 8 cores; per-box bilinear neighbor
indices (uint16 flat offsets into the [160*256] image) and lerp weights
(validity folded in) are host-precomputed input tensors. Per box:
  - 4 GPSIMD indirect_copy gathers pull the TL/TR/BL/BR neighbor stacks
    [128ch, 1024] out of the resident image,
  - 9 DVE tensor_tensor ops apply the separable weights via stride-0
    broadcast access patterns,
  - one DMA writes the [128, 32, 32] crop tile.

Because there is only ONE program, dispatch is a single jitted
shard_map(bass_exec) call across the 8 PJRT devices — one host->device
round trip per run() instead of eight.
"""

import sys
import threading

sys.path.insert(0, "/opt/trn_rl_repo")

import numpy as np

_B, _C, _H, _W, _N, _CROP = 4, 256, 160, 256, 256, 32
_CH = _C // 2  # channels per core
_HW = _H * _W  # flat image window per channel (40960 < 2**16: uint16 indexable)

# ---------------------------------------------------------------- compat ---
# This container's walrus accepts at most ONE semaphore sync-wait per
# instruction. Patch Tile's kernel-tail drain, and post-rewrite any
# instruction carrying N>1 waits into N-1 preceding single-wait
# EventSemaphore instructions on the same engine.
_ctr = [0]


def _apply_bass_patches():
    import bass_rust
    from concourse.tile import TileContext
    from concourse.vector_clock import ScopedClock

    def _drain_and_barrier_split_waits(self, tick_clock, wait_clock):
        nc = self.nc
        probe = nc.sync.nop()
        wait_clock.add_sem_waits(
            probe.ins, ScopedClock({None: tick_clock.global_clock})
        )
        si = probe.ins.sync_info
        waits = list(si.on_wait) if si is not None else []
        probe.ins.sync_info = None
        name_to_handle = dict(self.sems.allocated().items())
        for w in waits:
            h = name_to_handle.get(w.ant_name)
            if h is not None:
                nc.sync.wait_ge(h, w.wait_value)
            else:
                ev = nc.sync.nop()
                ev.ins.sync_info = bass_rust.SyncInfo(on_wait=[w], on_update=[])
        nc.sync.drain()
        nc.all_engine_barrier()
        popped = nc._tile_sem_poison_stack.pop()
        assert popped is self._sem_poison
        nc.clear_and_free_semaphores(list(self.sems.allocated().values()))
        nc.all_engine_barrier()

    TileContext._drain_and_barrier = _drain_and_barrier_split_waits


def _split_multi_waits(nc):
    import bass_rust
    import concourse.mybir as mybir

    for f in nc.m.functions:
        for bb in f.blocks:
            changed = False
            new = []
            for ins in bb.instructions:
                si = ins.sync_info
                if si is not None and si.on_wait and len(si.on_wait) > 1:
                    changed = True
                    waits = list(si.on_wait)
                    for w in waits[:-1]:
                        _ctr[0] += 1
                        new.append(
                            mybir.InstEventSemaphore(
                                name=f"I-wsplit-{_ctr[0]}",
                                engine=ins.engine,
                                ins=[],
                                outs=[],
                                sync_info=bass_rust.SyncInfo(
                                    on_wait=[w], on_update=[]
                                ),
                            )
                        )
                    ins.sync_info = bass_rust.SyncInfo(
                        on_wait=[waits[-1]], on_update=list(si.on_update)
                    )
                new.append(ins)
            if changed:
                bb.instructions = new


# ------------------------------------------------------------- host plan ---
def _plan(boxes, box_ind):
    """Mirror the reference's float32 sampling math exactly. Returns
    (K, slots, idx_arrs, wts_arrs) where idx_arrs[b] is [K, 128, 256] uint16
    (TL/TR/BL/BR neighbor indices, wrapped per 16-partition group) and
    wts_arrs[b] is [K, 128, 128] f32 (wt/wb/wl/wr, validity folded in);
    slots[b] is the list of original box ids for batch b's slots."""
    f32 = np.float32
    boxes = np.asarray(boxes, dtype=f32)
    box_ind = np.asarray(box_ind, dtype=np.int32)
    n_boxes = boxes.shape[0]
    y1, x1, y2, x2 = boxes[:, 0], boxes[:, 1], boxes[:, 2], boxes[:, 3]
    hs = (y2 - y1) * f32(_H - 1) / f32(_CROP - 1)
    ws = (x2 - x1) * f32(_W - 1) / f32(_CROP - 1)
    ii = np.arange(_CROP, dtype=f32)
    in_y = y1[:, None] * f32(_H - 1) + ii[None, :] * hs[:, None]  # [N, 32]
    in_x = x1[:, None] * f32(_W - 1) + ii[None, :] * ws[:, None]
    vy = (in_y >= 0) & (in_y <= _H - 1)
    vx = (in_x >= 0) & (in_x <= _W - 1)
    top_f = np.floor(in_y)
    left_f = np.floor(in_x)
    ly = (in_y - top_f).astype(f32)
    lx = (in_x - left_f).astype(f32)
    t = np.clip(top_f, 0, _H - 1).astype(np.int64)
    b = np.clip(top_f + 1, 0, _H - 1).astype(np.int64)
    l = np.clip(left_f, 0, _W - 1).astype(np.int64)
    r = np.clip(left_f + 1, 0, _W - 1).astype(np.int64)
    wt = np.where(vy, 1 - ly, 0).astype(f32)
    wb = np.where(vy, ly, 0).astype(f32)
    wl = np.where(vx, 1 - lx, 0).astype(f32)
    wr = np.where(vx, lx, 0).astype(f32)

    # flat neighbor indices [N, 32, 32] per quadrant
    TL = t[:, :, None] * _W + l[:, None, :]
    TR = t[:, :, None] * _W + r[:, None, :]
    BL = b[:, :, None] * _W + l[:, None, :]
    BR = b[:, :, None] * _W + r[:, None, :]
    quads = np.stack([TL, TR, BL, BR], axis=1).astype(np.uint16)  # [N,4,32,32]
    # wrap each 1024-list across 16 partitions: wrapped[p, s] = flat[s*16 + p%16]
    wrapped = quads.reshape(n_boxes, 4, 1024).reshape(n_boxes, 4, 64, 16)
    wrapped = np.transpose(wrapped, (0, 1, 3, 2))  # [N, 4, 16, 64]
    idx16 = wrapped.reshape(n_boxes, 4 * 16, 64)  # rows: q-major then p%16
    # build [N, 128, 256]: partition p, cols q*64:(q+1)*64 = wrapped[q, p%16]
    idx_full = np.zeros((n_boxes, 128, 256), dtype=np.uint16)
    for q in range(4):
        blk = wrapped[:, q]  # [N, 16, 64]
        idx_full[:, :, q * 64 : (q + 1) * 64] = np.tile(blk, (1, 8, 1))

    wts_full = np.zeros((n_boxes, 128, 128), dtype=f32)
    wts_full[:, :, 0:32] = wt[:, None, :]
    wts_full[:, :, 32:64] = wb[:, None, :]
    wts_full[:, :, 64:96] = wl[:, None, :]
    wts_full[:, :, 96:128] = wr[:, None, :]

    slots = [np.nonzero(box_ind == bb)[0] for bb in range(_B)]
    K = max(len(s) for s in slots)
    idx_arrs, wts_arrs = [], []
    for bb in range(_B):
        ia = np.zeros((K, 128, 256), dtype=np.uint16)
        wa = np.zeros((K, 128, 128), dtype=f32)
        ns = slots[bb]
        ia[: len(ns)] = idx_full[ns]
        wa[: len(ns)] = wts_full[ns]
        idx_arrs.append(ia)
        wts_arrs.append(wa)
    return K, slots, idx_arrs, wts_arrs


# ------------------------------------------------------- device program ---
def _build_program(K, split_waits=True):
    import concourse.bass as bass
    import concourse.mybir as mybir
    from concourse.tile import TileContext

    AL = mybir.AluOpType
    f32 = mybir.dt.float32
    u16 = mybir.dt.uint16
    nc = bass.Bass()
    img_p = nc.declare_dram_parameter("img", [128, _HW], f32, isOutput=False)
    idx_p = nc.declare_dram_parameter("idx", [K, 128, 256], u16, isOutput=False)
    wts_p = nc.declare_dram_parameter("wts", [K, 128, 128], f32, isOutput=False)
    out_p = nc.declare_dram_parameter(
        "out", [K, 128, _CROP * _CROP], f32, isOutput=True
    )
    with TileContext(nc) as tc:
        with (
            tc.tile_pool(name="imgp", bufs=1) as imgp,
            tc.tile_pool(name="meta", bufs=3) as metap,
            tc.tile_pool(name="gp", bufs=2) as gp,
            tc.tile_pool(name="op", bufs=2) as outp,
        ):
            IMG = imgp.tile([128, _HW], f32)
            nc.sync.dma_start(out=IMG[:], in_=img_p[:])
            for k in range(K):
                IDX = metap.tile([128, 256], u16, tag="idx")
                WTS = metap.tile([128, 128], f32, tag="wts")
                nc.sync.dma_start(out=IDX[:], in_=idx_p[k])
                nc.sync.dma_start(out=WTS[:], in_=wts_p[k])
                # num_valid_indices is ISA-capped at 1024 per IndirectCopy,
                # so the four neighbor stacks need four gathers.
                SALL = gp.tile([128, 4096], f32, tag="sall")
                S = [SALL[:, q * 1024 : (q + 1) * 1024] for q in range(4)]
                for q in range(4):
                    nc.gpsimd.indirect_copy(
                        S[q], IMG[:], IDX[:, q * 64 : (q + 1) * 64], True
                    )
                wt_b = WTS[:, 0:32].unsqueeze(2).broadcast_to((128, 32, 32))
                wb_b = WTS[:, 32:64].unsqueeze(2).broadcast_to((128, 32, 32))
                wl_b = WTS[:, 64:96].unsqueeze(1).broadcast_to((128, 32, 32))
                wr_b = WTS[:, 96:128].unsqueeze(1).broadcast_to((128, 32, 32))
                V = [s.rearrange("p (i j) -> p i j", j=_CROP) for s in S]
                vec = nc.vector
                vec.tensor_tensor(out=V[0], in0=V[0], in1=wl_b, op=AL.mult)
                vec.tensor_tensor(out=V[1], in0=V[1], in1=wr_b, op=AL.mult)
                vec.tensor_tensor(out=V[2], in0=V[2], in1=wl_b, op=AL.mult)
                vec.tensor_tensor(out=V[3], in0=V[3], in1=wr_b, op=AL.mult)
                vec.tensor_tensor(out=S[0], in0=S[0], in1=S[1], op=AL.add)
                vec.tensor_tensor(out=S[2], in0=S[2], in1=S[3], op=AL.add)
                vec.tensor_tensor(out=V[0], in0=V[0], in1=wt_b, op=AL.mult)
                vec.tensor_tensor(out=V[2], in0=V[2], in1=wb_b, op=AL.mult)
                OUT = outp.tile([128, 1024], f32, tag="out")
                vec.tensor_tensor(out=OUT[:], in0=S[0], in1=S[2], op=AL.add)
                nc.sync.dma_start(out=out_p[k], in_=OUT[:])
    if split_waits:
        _split_multi_waits(nc)
    return nc


# ------------------------------------------------------------- execution ---
class Runner:
    """Compiles ONE SPMD program for the (boxes, box_ind) plan; all 8 cores
    run it with per-core data. run() is a single jitted shard_map dispatch."""

    def __init__(self, image, boxes, box_ind):
        import jax
        from jax.sharding import Mesh, NamedSharding, PartitionSpec
        from jax.experimental.shard_map import shard_map
        import concourse.mybir as mybir
        from concourse import bass2jax

        _apply_bass_patches()
        bass2jax.install_neuronx_cc_hook()

        image = np.ascontiguousarray(np.asarray(image, dtype=np.float32))
        boxes = np.asarray(boxes, dtype=np.float32)
        box_ind = np.asarray(box_ind, dtype=np.int32)
        self.n_boxes = boxes.shape[0]

        K, slots, idx_arrs, wts_arrs = _plan(boxes, box_ind)
        self.K, self.slots = K, slots

        nc = _build_program(K)
        self.nc = nc

        devices = jax.devices()[:8]
        assert len(devices) == 8, devices

        partition_name = (
            nc.partition_id_tensor.name if nc.partition_id_tensor else None
        )
        in_names, out_names, out_avals, zero_outs = [], [], [], []
        for alloc in nc.m.functions[0].allocations:
            if not isinstance(alloc, mybir.MemoryLocationSet):
                continue
            name = alloc.memorylocations[0].name
            if alloc.kind == "ExternalInput":
                if name != partition_name:
                    in_names.append(name)
            elif alloc.kind == "ExternalOutput":
                out_names.append(name)
                shape = tuple(alloc.tensor_shape)
                dtype = mybir.dt.np(alloc.dtype)
                out_avals.append(jax.core.ShapedArray(shape, dtype))
                zero_outs.append(np.zeros(shape, dtype))
        n_params = len(in_names)
        all_names = in_names + out_names
        if partition_name is not None:
            all_names = all_names + [partition_name]
        self.out_names = out_names

        def _body(*args):
            operands = list(args)
            if partition_name is not None:
                operands.append(bass2jax.partition_id_tensor())
            outs = bass2jax._bass_exec_p.bind(
                *operands,
                out_avals=tuple(out_avals),
                in_names=tuple(all_names),
                out_names=tuple(out_names),
                lowering_input_output_aliases=(),
                sim_require_finite=False,
                sim_require_nnan=False,
                nc=nc,
            )
            return tuple(outs)

        mesh = Mesh(np.asarray(devices), ("core",))
        specs = (PartitionSpec("core"),) * (n_params + len(out_names))
        out_specs = (PartitionSpec("core"),) * len(out_names)
        self.jitted = jax.jit(
            shard_map(
                _body,
                mesh=mesh,
                in_specs=specs,
                out_specs=out_specs,
                check_rep=False,
            ),
            keep_unused=True,
        )

        # per-core input maps, concatenated to global arrays and placed once
        per_core = []
        for core in range(8):
            b, h = core // 2, core % 2
            in_map = {
                "img": np.ascontiguousarray(
                    image[b, h * _CH : (h + 1) * _CH].reshape(128, _HW)
                ),
                "idx": idx_arrs[b],
                "wts": wts_arrs[b],
            }
            per_core.append([in_map[n] for n in in_names])
        sh = NamedSharding(mesh, PartitionSpec("core"))
        self.args = [
            jax.device_put(
                np.concatenate([per_core[c][i] for c in range(8)], axis=0), sh
            )
            for i in range(n_params)
        ] + [
            jax.device_put(
                np.zeros((8 * z.shape[0], *z.shape[1:]), z.dtype), sh
            )
            for z in zero_outs
        ]
        # warmup: compile + NEFF load
        import jax as _jax

        _jax.block_until_ready(self.jitted(*self.args))

    def run(self):
        import jax

        outs = self.jitted(*self.args)
        jax.block_until_ready(outs)
        return outs

    def gather(self, outs):
        res = np.asarray(outs[0]).reshape(8, self.K, 128, _CROP, _CROP)
        out = np.zeros((self.n_boxes, _C, _CROP, _CROP), dtype=np.float32)
        for bb in range(_B):
            ns = self.slots[bb]
            for h in range(2):
                core = 2 * bb + h
                out[ns, h * _CH : (h + 1) * _CH] = res[core, : len(ns)]
        return out


def kernel(image, boxes, box_ind):
    r = Runner(image, boxes, box_ind)
    return r.gather(r.run())
